# revision 57
# baseline (speedup 1.0000x reference)
"""DenseGATConv-style GNN message passing kernel for Trainium2 (Bass/Tile).

Math (per graph b):
    e      = w_edge[edge_attr[b]]            # [N, N] gather from 4-entry table
    adj_w  = adj[b] * e                      # weighted adjacency
    agg    = adj_w @ x[b]                    # [N, C]
    out[b] = agg @ W_rel + b_rel + x[b] @ W_root

Key design points (v2):
  * Inputs are staged TRANSPOSED on the host (adj^T, edge_attr^T) so the
    elementwise chain produces adj_w^T tiles directly in [j-part, i-free]
    layout -- the aggregation matmul contracts over j on the PE with zero
    on-chip transposes (v1 spent ~72us/core on PE transposes).
  * The 4-entry gather w_edge[a], a in {0,1,2,3}, is fit EXACTLY by
        w_edge[a] = alpha * sin(beta*a + gamma) + delta
    (4 unknowns, 4 equation -- solved on host in fp64).  On device this is
    ONE ScalarE Sin activation + ONE VectorE scalar_tensor_tensor:
        s   = Sin(beta*a + gamma)            # ScalarE, uint8 in, fp16 out
        awT = (s + delta/alpha) * adjT       # DVE STT, 2x mode (16-bit)
    alpha is folded into W_rel on the host.
  * Everything on-chip runs in fp16 (adj in [0,1) and |e|<~1.4 are exactly
    representable to 2^-11): halves HBM traffic and hits the DVE 2x perf
    mode; edge_attr ships as uint8 (1B/elem instead of 4).
  * Output transform: out[i,c] = [aggT; xT].T @ [alpha*W_rel; W_root] + b_rel
    as one 128-contraction matmul per 128-row chunk; b_rel enters via a
    K=1 ones-row matmul that initializes the PSUM accumulator.

Sharding: data-parallel over batch B=16 across 8 cores (2 graphs/core);
weights replicated.
"""

import sys
from contextlib import ExitStack

sys.path.insert(0, "/opt/trn_rl_repo")

import numpy as np

_B, _N, _C = 16, 1024, 64
_NCORES = 8
_G = _B // _NCORES  # graphs per core
_P = 128
_NT = _N // _P  # 128-row tiles per graph
_H = 512  # half-graph columns (one PSUM bank of fp32)
_CHUNKS = [2, 3, 3]  # row-tiles per DMA/elementwise chunk (small first)

# Module-level knobs (test.py may flip these before calling kernel()).
TRACE = False
EA_DTYPE = "uint8"  # "uint8" | "float16" (fallback if u8 activation fails)
LAST_RESULTS = None  # BassKernelResults of the most recent run (for test.py)

_BUILD_CACHE = {}


def _pack_blob(adjT, eaT, eab):
    """Chunk-major byte blob: per chunk [adjT w*2 bytes | eaT w*eab bytes]."""
    B = adjT.shape[0]
    adj_b = adjT.view(np.uint8)  # [B, P, NT*N*2]
    ea_b = eaT.view(np.uint8)  # [B, P, NT*N*eab]
    segs = []
    off = 0
    for ctiles in _CHUNKS:
        w = ctiles * _N
        segs.append(adj_b[:, :, off * 2 : (off + w) * 2])
        segs.append(ea_b[:, :, off * eab : (off + w) * eab])
        off += w
    return np.ascontiguousarray(np.concatenate(segs, axis=2))


def _poly_coeffs(w_edge):
    """Cubic through (k, w_edge[k]) for k=0..3, float64. Returns c0..c3."""
    w = np.asarray(w_edge, dtype=np.float64).reshape(4)
    V = np.vander(np.arange(4.0), 4, increasing=True)
    return np.linalg.solve(V, w)


def _act_fit(f, w, n_starts=6000, seed=0):
    """Exact 4-point fit w[a] = alpha*f(beta*a+gamma)+delta via random-start
    Gauss-Newton (numpy only).  Returns (beta, gamma, alpha, delta) or None."""
    w = np.asarray(w, dtype=np.float64).reshape(4)
    a4 = np.arange(4.0)
    scale = max(np.max(np.abs(w)), 1e-30)
    rng = np.random.default_rng(seed)
    best = None
    for _ in range(n_starts):
        b = rng.uniform(-4.0, 4.0)
        g = rng.uniform(-8.0, 8.0)
        M = np.stack([f(b * a4 + g), np.ones(4)], axis=1)
        sol, *_ = np.linalg.lstsq(M, w, rcond=None)
        r = M @ sol - w
        v = float(r @ r)
        if best is None or v < best[0]:
            best = (v, b, g, float(sol[0]), float(sol[1]))
    p = np.array(best[1:], dtype=np.float64)
    eps = 1e-6
    for _ in range(200):
        b, g, al, de = p
        r = al * f(b * a4 + g) + de - w
        if np.abs(r).max() < 1e-12 * scale:
            break
        J = np.empty((4, 4))
        for j in range(4):
            q = p.copy()
            q[j] += eps
            J[:, j] = (q[2] * f(q[0] * a4 + q[1]) + q[3] - w - r) / eps
        try:
            step, *_ = np.linalg.lstsq(J, r, rcond=None)
        except np.linalg.LinAlgError:
            return None
        p = p - step
    b, g, al, de = p
    r = al * f(b * a4 + g) + de - w
    if np.abs(r).max() < 1e-9 * scale and abs(al) > 1e-9 * scale:
        return float(b), float(g), float(al), float(de)
    return None


def _fit_chain(w_edge):
    """Pick the device elementwise chain for e = w_edge[a], a in {0..3}.

    Preferred: exact silu fit  e = alpha*silu(beta*a+gamma) + delta
    (one ScalarE activation, unbounded domain, + tensor_scalar(+k) at 4x
    + tensor_tensor(*adjT) at 2x on the DVE).  Then sin (domain limited
    to [-pi,pi] on ScalarE, checked).  Falls back to the factored cubic.

    Returns (mode, params, lead): device computes awT = chain(a) * adjT
    such that true adj_w = lead * awT; `lead` is folded into W_rel.
    """
    w = np.asarray(w_edge, dtype=np.float64).reshape(4)
    v0, v1, v2, v3 = w
    scale = max(np.max(np.abs(w)), 1e-30)

    def silu(x):
        return x / (1.0 + np.exp(-np.clip(x, -60, 60)))

    fit = _act_fit(silu, w)
    if fit is not None and abs(fit[0]) * 3 + abs(fit[1]) < 30.0:
        b, g, al, de = fit
        return "silu", dict(beta=b, gamma=g, k=float(de / al)), al

    # sin fit: recurrence s_{k+1} + s_{k-1} = 2 cos(beta) s_k for s_k = v_k - d
    den = (v0 + v2) + 2.0 * v2 - (v1 + v3) - 2.0 * v1
    if abs(den) > 1e-9 * scale:
        d = ((v0 + v2) * v2 - (v1 + v3) * v1) / den
        if abs(v1 - d) > 1e-9 * scale:
            c = (v0 + v2 - 2.0 * d) / (2.0 * (v1 - d))
            if abs(c) < 1.0 - 1e-7:
                b = float(np.arccos(c))
                sb = np.sin(b)
                P = v0 - d  # alpha*sin(gamma)
                Q = ((v1 - d) - P * c) / sb  # alpha*cos(gamma)
                alpha = float(np.hypot(P, Q))
                g = float(np.arctan2(P, Q))
                args = b * np.arange(4.0) + g
                fitv = alpha * np.sin(args) + d
                if (
                    np.abs(fitv - w).max() < 1e-9 * scale
                    and alpha > 1e-9 * scale
                    and np.abs(args).max() <= np.pi  # ScalarE Sin domain
                ):
                    return (
                        "sin",
                        dict(beta=b, gamma=g, k=float(d / alpha)),
                        alpha,
                    )

    c0, c1, c2, c3 = _poly_coeffs(w)
    tol = 1e-7 * scale
    if abs(c3) > tol:
        # monic cubic a^3+A a^2+B a+C = (a-r)((a+h)^2 + v2)
        A, Bc, Cc = c2 / c3, c1 / c3, c0 / c3
        roots = np.roots([1.0, A, Bc, Cc])
        r = float(np.real(roots[np.argmin(np.abs(np.imag(roots)))]))
        p = A + r
        q = Bc + p * r
        return "cubic", dict(r=r, h=p / 2.0, v2=q - p * p / 4.0), c3
    if abs(c2) > tol:
        p2, q2 = c1 / c2, c0 / c2
        return "quad", dict(h=p2 / 2.0, v2=q2 - p2 * p2 / 4.0), c2
    if abs(c1) > tol:
        return "linear", dict(r=-c0 / c1), c1
    return "const", dict(), c0


def _emit_graph(nc, pools, g, dram, mode, params, ea_is_u8):
    from concourse import mybir

    OP = mybir.AluOpType
    AF = mybir.ActivationFunctionType
    f32 = pools["f32"]
    f16 = pools["f16"]
    blob_d, x_d, xT_d, out_d = (
        dram["blob"], dram["x"], dram["xT"], dram["out"],
    )

    bf16 = pools["bf16"]

    import contextlib

    # fused adjT+eaT chunk DMAs first so they lead the sequencer queues:
    # blob chunk layout per partition = [adjT w*2 bytes | eaT w bytes].
    # First chunks are single tiles so the pipeline starts sooner.
    eab = 1 if ea_is_u8 else 2
    n_chunks = len(_CHUNKS)
    ea_tiles, adj_tiles = [], []
    off = 0
    for ch, ctiles in enumerate(_CHUNKS):
        w = ctiles * _N
        cb = w * (2 + eab)
        hot = g == 0 and ch < 2
        prio = pools["tc"].high_priority() if hot else contextlib.nullcontext()
        with prio:
            blob_t = pools["blobp"].tile([_P, cb], pools["u8"], name="blob_t")
            # alternate the two HWDGE rings (SP / ACT): each ring executes
            # its DMAs in FIFO order, so one ring serializes the stream
            eng = nc.sync if (ch % 2 == 0) else nc.scalar
            eng.dma_start(out=blob_t[:], in_=blob_d[g, :, off : off + cb])
        off += cb
        adj_tiles.append(blob_t[:, 0 : 2 * w].bitcast(f16))
        ea_tiles.append(
            blob_t[:, 2 * w : cb]
            if ea_is_u8
            else blob_t[:, 2 * w : cb].bitcast(f16)
        )

    # x in lhsT layout (host pre-tiled): xs[p, t*C+c] = x[t*128+p, c]
    # (gpsimd SWDGE: keeps the Scalar queue clear for the silu stream)
    xs = pools["xsp"].tile([_P, _NT * _C], bf16)
    nc.gpsimd.dma_start(out=xs[:], in_=x_d[g, :, :])

    # stacked lhsT for the output transform: rows 0:64 <- aggT (later),
    # rows 64:128 <- xT (DMA now)
    stk = []
    for half in range(2):
        t = pools["stkp"].tile([_P, _H], bf16, tag=f"stk{half}")
        nc.gpsimd.dma_start(
            out=t[_C : 2 * _C, :],
            in_=xT_d[g, :, half * _H : (half + 1) * _H],
        )
        stk.append(t)

    p_agg = [
        pools["ps_agg"].tile(
            [_C, _H], f32, tag=f"ps_agg{half}", name=f"p_agg{half}"
        )
        for half in range(2)
    ]

    jt_base = 0
    for ch, ctiles in enumerate(_CHUNKS):
        CW = ctiles * _N
        eaT_a, adjT_a = ea_tiles[ch], adj_tiles[ch]
        awT_t = pools["awp"].tile([_P, CW], bf16)
        if mode in ("silu", "sin"):
            s_t = pools["sp"].tile([_P, CW], f16)
            nc.scalar.activation(
                s_t[:], eaT_a,
                AF.Silu if mode == "silu" else AF.Sin,
                bias=pools["abias_sb"][:, 0:1], scale=float(params["beta"]),
            )
            # s += k in place at 4x, awT = s * adjT at 2x
            # (scalar_tensor_tensor has no 2x uop -- always 1x)
            nc.vector.tensor_scalar(
                s_t[:], s_t[:], float(params["k"]), None, OP.add
            )
            nc.vector.tensor_tensor(awT_t[:], s_t[:], adjT_a, OP.mult)
        elif mode == "cubic":
            s_t = pools["sp"].tile([_P, CW], f16)
            nc.scalar.activation(
                s_t[:], eaT_a, AF.Square,
                bias=pools["abias_sb"][:, 0:1], scale=1.0,
            )
            qt_t = pools["qtp"].tile([_P, CW], f16)
            nc.vector.scalar_tensor_tensor(
                qt_t[:], eaT_a, float(params["r"]), adjT_a,
                OP.subtract, OP.mult,
            )
            nc.vector.scalar_tensor_tensor(
                awT_t[:], s_t[:], float(params["v2"]), qt_t[:], OP.add, OP.mult
            )
        elif mode == "quad":
            s_t = pools["sp"].tile([_P, CW], f16)
            nc.scalar.activation(
                s_t[:], eaT_a, AF.Square,
                bias=pools["abias_sb"][:, 0:1], scale=1.0,
            )
            nc.vector.scalar_tensor_tensor(
                awT_t[:], s_t[:], float(params["v2"]), adjT_a, OP.add, OP.mult
            )
        elif mode == "linear":
            nc.vector.scalar_tensor_tensor(
                awT_t[:], eaT_a, float(params["r"]), adjT_a,
                OP.subtract, OP.mult,
            )
        else:  # const
            nc.vector.tensor_copy(awT_t[:], adjT_a)

        # accumulate agg^T[c, i] += x_jt^T @ awT_jt for both halves
        for sub in range(ctiles):
            jt = jt_base + sub
            for half in range(2):
                nc.tensor.matmul(
                    p_agg[half][:],
                    lhsT=xs[:, jt * _C : (jt + 1) * _C],
                    rhs=awT_t[:, sub * _N + half * _H : sub * _N + (half + 1) * _H],
                    start=(jt == 0),
                    stop=(jt == _NT - 1),
                )
        jt_base += ctiles

    # output transform per half.  g0's aggT copies land mid-kernel -- keep
    # them OFF the Scalar engine so they don't stall the silu stream; g1's
    # land after the last silu, when Scalar is idle.
    for half in range(2):
        if g == 0:
            nc.vector.tensor_copy(stk[half][:_C, :], p_agg[half][:])
        else:
            nc.scalar.copy(out=stk[half][:_C, :], in_=p_agg[half][:])
        outb = pools["outp"].tile([_P, 4 * _C], f32)
        p_out = pools["ps_out"].tile([_P, 4 * _C], f32, tag="ps_out")
        for q in range(4):
            nc.tensor.matmul(
                p_out[:, q * _C : (q + 1) * _C],
                lhsT=stk[half][:, q * _P : (q + 1) * _P],
                rhs=pools["wstack_sb"][:, :],
                start=True, stop=True,
            )
        # copy + b_rel add in one pass (brelf is host-pre-broadcast x4)
        nc.vector.tensor_tensor(
            outb[:], p_out[:], pools["brelf_sb"][:, :], OP.add
        )
        # out dram is host-staged as [G, 2, 128, 4C]: direct 2D store
        nc.sync.dma_start(out=out_d[g, half, :, :], in_=outb[:])


def _build_module(mode, params, ea_is_u8):
    import concourse.bass as bass  # noqa: F401
    from concourse import bacc, mybir
    from concourse.tile import TileContext

    f32 = mybir.dt.float32
    f16 = mybir.dt.float16
    bf16 = mybir.dt.bfloat16
    u8 = mybir.dt.uint8

    nc = bacc.Bacc(
        "TRN2", target_bir_lowering=False, debug=False, num_devices=_NCORES
    )

    # adjT+eaT ship as one host-packed byte blob per graph, pre-tiled to
    # [128 partitions, chunk-major contiguous bytes]: one fat DMA per chunk.
    ea_bytes = 1 if ea_is_u8 else 2
    blob_w = _NT * _N * (2 + ea_bytes)
    dram = {
        "blob": nc.dram_tensor("blob", [_G, _P, blob_w], u8, kind="ExternalInput"),
        "x": nc.dram_tensor("x", [_G, _P, _NT * _C], bf16, kind="ExternalInput"),
        "xT": nc.dram_tensor("xT", [_G, _C, _N], bf16, kind="ExternalInput"),
        "wstack": nc.dram_tensor(
            "wstack", [2 * _C, _C], bf16, kind="ExternalInput"
        ),
        "brelf": nc.dram_tensor(
            "brelf", [_P, 4 * _C], bf16, kind="ExternalInput"
        ),
        "out": nc.dram_tensor(
            "out", [_G, 2, _P, 4 * _C], f32, kind="ExternalOutput"
        ),
    }

    pool_specs = [
        ("consts", 1, None),
        ("blobp", 8, None),
        ("sp", 6, None),
        ("qtp", 2, None),
        ("awp", 6, None),
        ("xsp", 2, None),
        ("stkp", 4, None),
        ("outp", 2, None),
        ("ps_agg", 2, "PSUM"),
        ("ps_out", 2, "PSUM"),
    ]

    with TileContext(nc) as tc, ExitStack() as ctx:
        pools = {"f32": f32, "f16": f16, "bf16": bf16, "u8": u8, "tc": tc}
        for name, bufs, space in pool_specs:
            kw = {"space": space} if space else {}
            pools[name] = ctx.enter_context(tc.tile_pool(name=name, bufs=bufs, **kw))

        # consts go through gpsimd (SWDGE) so the first graph's chunk
        # DMAs lead the sync/scalar HWDGE queues
        wstack = pools["consts"].tile([2 * _C, _C], bf16, tag="wstack")
        nc.gpsimd.dma_start(out=wstack[:], in_=dram["wstack"][:, :])
        pools["wstack_sb"] = wstack
        brelf = pools["consts"].tile([_P, 4 * _C], bf16, tag="brelf")
        nc.gpsimd.dma_start(out=brelf[:], in_=dram["brelf"][:, :])
        pools["brelf_sb"] = brelf

        # per-partition activation bias (gamma for Silu/Sin, h for Square)
        if mode in ("silu", "sin", "cubic", "quad"):
            ab = pools["consts"].tile([_P, 1], f32, tag="abias")
            bias_val = params["gamma"] if mode in ("silu", "sin") else params["h"]
            nc.vector.memset(ab[:], float(bias_val))
            pools["abias_sb"] = ab

        if mode == "silu":
            # dummy Silu on a [128,1] tile as the FIRST Scalar instruction:
            # pulls the silu_and_others ACT_TABLE_LOAD into the startup
            # dead-time instead of serializing before the first real chunk
            dum = pools["consts"].tile([_P, 1], f32, tag="dummy_act")
            nc.scalar.activation(dum[:], ab[:], mybir.ActivationFunctionType.Silu)

        for g in range(_G):
            _emit_graph(nc, pools, g, dram, mode, params, ea_is_u8)

    nc.finalize()
    return nc


def _get_module(w_edge, ea_dtype):
    mode, params, lead = _fit_chain(w_edge)
    ea_is_u8 = ea_dtype == "uint8" and mode in ("silu", "sin", "quad", "const")
    # cubic/linear read ea on the DVE -> needs a float dtype
    key = (
        mode,
        tuple(sorted((k, round(v, 15)) for k, v in params.items())),
        ea_is_u8,
    )
    if key not in _BUILD_CACHE:
        _BUILD_CACHE[key] = _build_module(mode, params, ea_is_u8)
    return _BUILD_CACHE[key], lead, ea_is_u8


def kernel(x, adj, edge_attr, W_rel, b_rel, W_root, w_edge):
    global LAST_RESULTS
    from concourse import mybir
    from concourse.bass_utils import run_bass_kernel_spmd

    f16np = mybir.dt.np(mybir.dt.float16)
    bf16np = mybir.dt.np(mybir.dt.bfloat16)

    x = np.asarray(x, dtype=np.float32)
    adj = np.asarray(adj, dtype=np.float32)
    ea = np.asarray(edge_attr, dtype=np.int32).reshape(_B, _N, _N)
    W_rel = np.asarray(W_rel, dtype=np.float64)
    W_root = np.asarray(W_root, dtype=np.float64)
    b_rel = np.asarray(b_rel, dtype=np.float64).reshape(1, _C)
    w_edge = np.asarray(w_edge)

    nc, lead, ea_is_u8 = _get_module(w_edge, EA_DTYPE)

    def tile_rows(a):
        """[B, N, F] -> [B, 128, NT*F]: row j*128+p of graph b lands at
        [b, p, j*F:(j+1)*F] -- one contiguous free-dim line per partition."""
        B, N, F = a.shape
        return np.ascontiguousarray(
            a.reshape(B, _NT, _P, F).transpose(0, 2, 1, 3).reshape(B, _P, _NT * F)
        )

    adjT = tile_rows(np.ascontiguousarray(adj.transpose(0, 2, 1))).astype(f16np)
    eaT = tile_rows(np.ascontiguousarray(ea.transpose(0, 2, 1)))
    eaT = eaT.astype(np.uint8) if ea_is_u8 else eaT.astype(f16np)
    blob = _pack_blob(adjT, eaT, 1 if ea_is_u8 else 2)
    x16 = tile_rows(x).astype(bf16np)
    xT = np.ascontiguousarray(x.transpose(0, 2, 1)).astype(bf16np)
    wstack = np.ascontiguousarray(
        np.concatenate([lead * W_rel, W_root], axis=0)
    ).astype(bf16np)
    brelf = np.ascontiguousarray(
        np.broadcast_to(np.tile(b_rel, (1, 4)), (_P, 4 * _C))
    ).astype(bf16np)

    in_maps = []
    for c in range(_NCORES):
        sl = slice(c * _G, (c + 1) * _G)
        in_maps.append(
            {
                "blob": blob[sl],
                "x": x16[sl],
                "xT": xT[sl],
                "wstack": wstack,
                "brelf": brelf,
            }
        )

    res = run_bass_kernel_spmd(nc, in_maps, list(range(_NCORES)), trace=TRACE)
    LAST_RESULTS = res
    # out is staged [G, 2, 128, 4*C]: row i = half*512 + q*128 + p
    outs = np.concatenate(
        [np.asarray(res.results[c]["out"]) for c in range(_NCORES)], axis=0
    ).astype(np.float32)
    out = (
        outs.reshape(_B, 2, _P, 4, _C)
        .transpose(0, 1, 3, 2, 4)
        .reshape(_B, _N, _C)
    )
    return np.ascontiguousarray(out)
